# revision 17
# baseline (speedup 1.0000x reference)
"""EdgeAttentionAggregator Trainium2 kernel (8-core SPMD).

Reference computation (per node n, K=32 neighbors, D=128 out dim, E=64 edge):
    x = features @ W                                    [N, D]
    e[n,k]   = leakyrelu(x[n]@a_self + x[u]@a_nb + emb[n,k]@a_edge),  u=neigh[n,k]
    att      = softmax_k(e)
    h[n]     = sum_k att[n,k] * x[neigh[n,k]]
    h_e[n]   = sum_k att[n,k] * emb[n,k]
    out      = elu([x | h | h_e])                       [N, 2D+E]

Distribution: nodes sharded over 8 cores. Each core projects its shard,
a chunked AllGather replicates a PAIR-row table into every core's DRAM
(overlapping projection), and each core resolves neighbor reads with
dma_gather (mlp GPSIMD library) over 4 SWDGE queues.

The gather phase is descriptor-generation bound (one descriptor per edge,
Q7 core-pair per queue), so rows are packed to 512 bytes per pair:
  per node-half (256B): [hi: fp8e4m3(x@R) x128 | lo: fp8e4m3 residual
  dims 0:124 | s = x@a_nb as f32]
R is a host-chosen random rotation; quantization error of the 4 dims that
lack a residual is spread across all output dims, and the rotation is
undone for free by using R^T instead of the identity in the final PE
transpose of h. s rides exactly (f32) in the row.

Per-tile pipeline (packed edge layout: stream pos g*128 + 32*ns + k holds
edge (node 32*ns+g, k)): s_nb blended from the two parity s-slots, s_edge
on DVE in packed layout, one 32x32 block transpose to node-major for the
leakyrelu/softmax (ACT), block-diagonal parity-masked attention matrices,
h^T and h_e^T accumulated on the PE.

elu(v) = max(v,0) + exp(min(v,0)) - 1; lrelu(v) = 0.6v + 0.4|v| (slope 0.2).
"""

import numpy as np
from contextlib import ExitStack

import concourse.bass as bass
import concourse.tile as tile
from concourse import bacc, mybir
from concourse.tile import add_dep_helper
from concourse.bass_utils import run_bass_kernel_spmd
from concourse.masks import make_identity
from concourse import library_config

F32 = mybir.dt.float32
I16 = mybir.dt.int16
BF16 = mybir.dt.bfloat16
FP8 = mybir.dt.float8e4
AF = mybir.ActivationFunctionType
OP = mybir.AluOpType

ALPHA = 0.2   # leaky relu slope
CHUNK = 1024  # max dma_gather indices per call on this runtime
LO = 124      # residual-covered dims per node


class Cfg:
    def __init__(self, n_total=50000, k=32, in_dim=256, d=128, e=64, ncores=8):
        assert n_total % ncores == 0
        self.n_total = n_total
        self.k = k
        self.in_dim = in_dim
        self.d = d
        self.e = e
        self.ncores = ncores
        self.shard = n_total // ncores
        self.tiles = (self.shard + 127) // 128
        self.shard_pad = self.tiles * 128
        self.pairs = self.shard_pad // 2
        self.tbl_pairs = ncores * self.pairs
        assert self.tbl_pairs <= 32767
        self.row = 512            # fp8 units (bytes) per pair row
        self.half_row = 256
        self.sh_cols = 2 * d + 3  # f32: [x | ssl06 | ssl04 | x' | s_nb]
        self.out_cols = 2 * d + e
        self.nsub = 128 // k
        self.per_tile_idx = 128 * k
        self.chunks = self.per_tile_idx // CHUNK
        self.idx_cols = self.per_tile_idx // 16
        # AllGather chunk boundaries in pair rows (13/12/12/12 tiles)
        self.agb = [0, 832, 1600, 2368, self.pairs]
        assert all(b % 64 == 0 for b in self.agb)


def build(cfg: Cfg):
    c = cfg
    nc = bacc.Bacc("TRN2", target_bir_lowering=False, debug=False,
                   num_devices=c.ncores, num_swdge_queues=4)

    feat = nc.dram_tensor("feat", [c.shard_pad, c.in_dim], BF16,
                          kind="ExternalInput").ap()
    wext = nc.dram_tensor("wext", [c.in_dim, c.sh_cols], BF16,
                          kind="ExternalInput").ap()
    embd = nc.dram_tensor("embd", [c.shard_pad, c.k * c.e], BF16,
                          kind="ExternalInput").ap()
    aer = nc.dram_tensor("aer", [128, c.k * c.e], BF16,
                         kind="ExternalInput").ap()
    msk = nc.dram_tensor("msk", [128, 128], BF16, kind="ExternalInput").ap()
    rtb = nc.dram_tensor("rtb", [128, 128], BF16, kind="ExternalInput").ap()
    idx = nc.dram_tensor("idx", [c.tiles * 128, c.idx_cols], I16,
                         kind="ExternalInput").ap()
    parp = nc.dram_tensor("parp", [c.tiles * 128, 2 * c.k], BF16,
                          kind="ExternalInput").ap()
    outd = nc.dram_tensor("outd", [c.shard_pad, c.out_cols], F32,
                          kind="ExternalOutput").ap()
    shard_pair = nc.dram_tensor("shard_pair", [c.pairs, c.row // 2], BF16).ap()
    table = nc.dram_tensor("table", [c.tbl_pairs, c.row // 2], BF16).ap()

    with tile.TileContext(nc) as tc:
        _body(tc, c, feat, wext, embd, aer, msk, rtb, idx, parp, outd,
              shard_pair, table)

    nc.compile()
    return nc


def _body(tc, c: Cfg, feat, wext, embd, aer, msk, rtb, idx, parp, outd,
          shard_pair, table):
    nc = tc.nc
    D, K, E = c.d, c.k, c.e
    KE = K * E
    HR = c.half_row

    with ExitStack() as ctx:
        const = ctx.enter_context(tc.tile_pool(name="const", bufs=1))

        ident = const.tile([128, 128], F32, tag="ident")
        make_identity(nc, ident[:])
        identb = const.tile([128, 128], BF16, tag="identb")
        nc.vector.tensor_copy(identb[:], ident[:])
        rt_sb = const.tile([128, 128], BF16, tag="rt")
        nc.sync.dma_start(rt_sb[:], rtb[:, :])

        w_sb = []
        for ci in range(c.in_dim // 128):
            w = const.tile([128, c.sh_cols], BF16, tag=f"w{ci}")
            nc.sync.dma_start(w[:], wext[ci * 128:(ci + 1) * 128, :])
            w_sb.append(w)

        aer_sb = const.tile([128, KE], BF16, tag="aer")
        nc.sync.dma_start(aer_sb[:], aer[:, :])
        msk_sb = const.tile([128, 128], BF16, tag="msk")
        nc.sync.dma_start(msk_sb[:], msk[:, :])

        # resident per-tile f32 [x | ssl06 | ssl04] (130 cols per tile)
        xres = const.tile([128, c.tiles * 130], F32, tag="xres")

        n_sh = 3
        shtiles = [const.tile([128, HR], FP8, tag=f"sh{i}", name=f"sh{i}")
                   for i in range(n_sh)]

        lib = nc.gpsimd.load_library(library_config.mlp)

        # -------- Phase A: project own shard --------
        shard_writes = []
        with ExitStack() as actx:
            pa = actx.enter_context(tc.tile_pool(name="pa", bufs=3))
            psa = actx.enter_context(
                tc.tile_pool(name="psa", bufs=3, space="PSUM"))
            for t in range(c.tiles):
                ft = pa.tile([128, c.in_dim], BF16, tag="ft")
                nc.sync.dma_start(ft[:], feat[t * 128:(t + 1) * 128, :])
                ps_x = psa.tile([128, c.sh_cols], F32, tag="ps_x")
                nchunks = c.in_dim // 128
                for ci in range(nchunks):
                    ps_t = psa.tile([128, 128], BF16, tag="ps_t")
                    nc.tensor.transpose(ps_t[:], ft[:, ci * 128:(ci + 1) * 128],
                                        identb[:])
                    fT = pa.tile([128, 128], BF16, tag=f"fT{ci}")
                    nc.scalar.copy(fT[:], ps_t[:])
                    nc.tensor.matmul(ps_x[:], lhsT=fT[:], rhs=w_sb[ci][:],
                                     start=(ci == 0), stop=(ci == nchunks - 1))
                nc.vector.tensor_copy(xres[:, t * 130:(t + 1) * 130],
                                      ps_x[:, 0:130])
                sh = shtiles[t % n_sh]
                # staging row per node: [hi fp8 x128 | lo fp8 x124 | s f32]
                nc.vector.tensor_copy(sh[:, 0:D], ps_x[:, 130:130 + D])
                nc.vector.tensor_tensor(out=sh[:, D:D + LO],
                                        in0=ps_x[:, 130:130 + LO],
                                        in1=sh[:, 0:LO], op=OP.subtract)
                shb = sh[:].bitcast(BF16)
                nc.vector.tensor_copy(shb[:, 126:127],
                                      ps_x[:, c.sh_cols - 1:c.sh_cols])
                nc.vector.tensor_tensor(out=shb[:, 127:128],
                                        in0=ps_x[:, c.sh_cols - 1:c.sh_cols],
                                        in1=shb[:, 126:127], op=OP.subtract)
                wr = nc.sync.dma_start(
                    shard_pair[t * 64:(t + 1) * 64, :]
                    .rearrange("r (p q) -> r p q", p=2),
                    sh[:].bitcast(BF16))
                shard_writes.append(wr)

        # -------- AllGather the pair-row table --------
        if c.ncores > 1:
            cc = nc.gpsimd.collective_compute(
                "AllGather", OP.bypass,
                replica_groups=[list(range(c.ncores))],
                ins=[shard_pair[:, :]],
                outs=[table[:, :]],
            )
        else:
            cc = nc.sync.dma_start(table[:, :], shard_pair[:, :])
        for wr in shard_writes:
            add_dep_helper(cc.ins, wr.ins, reason="table after shard write")
        ccs = [cc]

        # -------- Phase B: attention + aggregation --------
        pb = ctx.enter_context(tc.tile_pool(name="pb", bufs=3))
        psb = ctx.enter_context(tc.tile_pool(name="psb", bufs=2, space="PSUM"))

        for t in range(c.tiles):
            r0, r1 = t * 128, (t + 1) * 128
            idxt = pb.tile([128, c.idx_cols], I16, tag="idxt")
            nc.sync.dma_start(idxt[:], idx[r0:r1, :])
            part = pb.tile([128, 2 * K], BF16, tag="part")
            nc.sync.dma_start(part[:], parp[r0:r1, :])
            embt = pb.tile([128, KE], BF16, tag="embt")
            nc.sync.dma_start(embt[:], embd[r0:r1, :])

            gx = pb.tile([128, K * c.row // 2], BF16, tag="gx")
            nb_per = CHUNK // 128
            for ci in range(c.chunks):
                g1 = nc.gpsimd.dma_gather(
                    out_ap=gx[:, ci * nb_per * c.row // 2:
                              (ci + 1) * nb_per * c.row // 2]
                    .rearrange("p (b e) -> p b e", e=c.row // 2),
                    in_ap=table,
                    idxs_ap=idxt[:, ci * (CHUNK // 16):(ci + 1) * (CHUNK // 16)],
                    num_idxs=CHUNK,
                    num_idxs_reg=CHUNK,
                    elem_size=c.row // 2,
                    queue_num=(t * c.chunks + ci) % 4,
                )
                for cc in ccs:
                    add_dep_helper(g1.ins, cc.ins, reason="gather after table")
                add_dep_helper(g1.ins, lib.ins, reason="gather after lib")

            # s_nb: parity blend of the bf16 hi/lo s slots
            gxb = gx[:].rearrange("p (g w) -> p g w", w=HR)
            par_pk = part[:, 0:K]
            sev = pb.tile([128, K], F32, tag="sev")
            nc.vector.tensor_tensor(out=sev[:].unsqueeze(2),
                                    in0=gxb[:, :, 126:127],
                                    in1=gxb[:, :, 127:128], op=OP.add)
            sod = pb.tile([128, K], F32, tag="sod")
            nc.vector.tensor_tensor(out=sod[:].unsqueeze(2),
                                    in0=gxb[:, :, 254:255],
                                    in1=gxb[:, :, 255:256], op=OP.add)
            sdiff = pb.tile([128, K], F32, tag="sdiff")
            nc.vector.tensor_tensor(out=sdiff[:], in0=sod[:], in1=sev[:],
                                    op=OP.subtract)
            sdp = pb.tile([128, K], F32, tag="sdp")
            nc.vector.tensor_tensor(out=sdp[:], in0=sdiff[:], in1=par_pk,
                                    op=OP.mult)
            spk = pb.tile([128, K], F32, tag="spk")
            nc.vector.tensor_tensor(out=spk[:], in0=sev[:], in1=sdp[:],
                                    op=OP.add)

            # s_edge (packed layout): sum_e embP[p, g*64+e] * a_edge[e]
            prod = pb.tile([128, KE], BF16, tag="prod")
            nc.vector.tensor_tensor(out=prod[:], in0=embt[:], in1=aer_sb[:],
                                    op=OP.mult)
            sed = pb.tile([128, K], F32, tag="sed")
            nc.vector.tensor_reduce(
                out=sed[:], in_=prod[:].rearrange("p (k e) -> p k e", k=K),
                axis=mybir.AxisListType.X, op=OP.add)

            epk = pb.tile([128, K], F32, tag="epk")
            nc.vector.tensor_tensor(out=epk[:], in0=spk[:], in1=sed[:],
                                    op=OP.add)
            enm = pb.tile([128, K], F32, tag="enm")
            nc.vector.transpose(enm[:], epk[:])   # packed -> node-major

            # e = lrelu(v + s_self) = 0.6(v+s) + 0.4|v+s|
            ssl06 = xres[:, t * 130 + D: t * 130 + D + 1]
            ssl04 = xres[:, t * 130 + D + 1: t * 130 + D + 2]
            ab = pb.tile([128, K], F32, tag="ab")
            nc.scalar.activation(ab[:], enm[:], AF.Abs, bias=ssl04,
                                 scale=ALPHA * 2)
            e6 = pb.tile([128, K], F32, tag="e6")
            nc.vector.tensor_scalar(out=e6[:], in0=enm[:],
                                    scalar1=1.0 - ALPHA * 2, scalar2=ssl06,
                                    op0=OP.mult, op1=OP.add)
            elog = pb.tile([128, K], F32, tag="elog")
            nc.vector.tensor_tensor(out=elog[:], in0=e6[:], in1=ab[:],
                                    op=OP.add)

            p = pb.tile([128, K], BF16, tag="p")
            den = pb.tile([128, 1], F32, tag="den")
            nc.scalar.activation(p[:], elog[:], AF.Exp, accum_out=den[:])
            inv = pb.tile([128, 1], F32, tag="inv")
            nc.vector.reciprocal(inv[:], den[:])

            ppk = pb.tile([128, K], BF16, tag="ppk")
            nc.vector.transpose(ppk[:], p[:])     # node-major -> packed
            asb = pb.tile([128, 128], BF16, tag="asb")
            nc.vector.tensor_tensor(
                out=asb[:],
                in0=ppk[:].unsqueeze(2).to_broadcast([128, K, c.nsub]),
                in1=msk_sb[:], op=OP.mult)
            aev = pb.tile([128, 128], BF16, tag="aev")
            nc.vector.tensor_tensor(
                out=aev[:], in0=asb[:],
                in1=part[:, K:2 * K].unsqueeze(2).to_broadcast([128, K, c.nsub]),
                op=OP.mult)
            aod = pb.tile([128, 128], BF16, tag="aod")
            nc.vector.tensor_tensor(
                out=aod[:], in0=asb[:],
                in1=par_pk.unsqueeze(2).to_broadcast([128, K, c.nsub]),
                op=OP.mult)

            # h'^T on the PE: per block g accumulate hi/lo, even/odd
            # reconstruct x = hi + lo in bf16 once (DVE), then 2 matmuls/block
            gx8v = gx[:].bitcast(FP8).rearrange("p (b e) -> p b e", e=HR)
            xf = pb.tile([128, 2 * K * D], BF16, tag="xf")
            xfv = xf[:].rearrange("p (b e) -> p b e", e=D)
            nc.vector.tensor_tensor(out=xfv[:, :, 0:LO],
                                    in0=gx8v[:, :, 0:LO],
                                    in1=gx8v[:, :, D:D + LO], op=OP.add)
            nc.vector.tensor_copy(xfv[:, :, LO:D], gx8v[:, :, LO:D])
            htps = psb.tile([128, 128], F32, tag="htps")
            for g in range(K):
                o = g * c.nsub
                b0 = 2 * g * D
                nc.tensor.matmul(htps[:, o:o + c.nsub],
                                 lhsT=xf[:, b0:b0 + D],
                                 rhs=aev[:, o:o + c.nsub],
                                 start=True, stop=False)
                nc.tensor.matmul(htps[:, o:o + c.nsub],
                                 lhsT=xf[:, b0 + D:b0 + 2 * D],
                                 rhs=aod[:, o:o + c.nsub],
                                 start=False, stop=True)
            htsb = pb.tile([128, 128], BF16, tag="htsb")
            nc.scalar.copy(htsb[:].rearrange("p (m g) -> p m g", m=c.nsub),
                           htps[:].rearrange("p (g m) -> p m g", m=c.nsub))
            # un-rotate while transposing: h = (h'^T)^T @ R^T
            hps = psb.tile([128, D], F32, tag="hps")
            nc.tensor.matmul(hps[:], lhsT=htsb[:], rhs=rt_sb[:],
                             start=True, stop=True)

            # h_e^T on the PE: per block g, emb columns vs asb
            hetps = psb.tile([64, 128], F32, tag="hetps")
            for g in range(K):
                o = g * c.nsub
                nc.tensor.matmul(hetps[:, o:o + c.nsub],
                                 lhsT=embt[:, g * E:(g + 1) * E],
                                 rhs=asb[:, o:o + c.nsub],
                                 start=True, stop=True)
            hetsb = pb.tile([64, 128], BF16, tag="hetsb")
            nc.scalar.copy(hetsb[:].rearrange("p (m g) -> p m g", m=c.nsub),
                           hetps[:].rearrange("p (g m) -> p m g", m=c.nsub))
            heps = psb.tile([128, E], BF16, tag="heps")
            nc.tensor.transpose(heps[:], hetsb[:], identb[0:64, 0:64])

            vt = pb.tile([128, c.out_cols], F32, tag="vt")
            nc.scalar.copy(vt[:, 0:D], xres[:, t * 130:t * 130 + D])
            nc.scalar.activation(vt[:, D:2 * D], hps[:], AF.Copy, bias=0.0,
                                 scale=inv[:])
            nc.scalar.activation(vt[:, 2 * D:], heps[:], AF.Copy, bias=0.0,
                                 scale=inv[:])

            # elu(v) = max(v,0) + exp(-relu(-v)) - 1
            mn = pb.tile([128, c.out_cols], F32, tag="mn")
            nc.scalar.activation(mn[:], vt[:], AF.Relu, scale=-1.0)
            ex = pb.tile([128, c.out_cols], F32, tag="ex")
            nc.scalar.activation(ex[:], mn[:], AF.Exp, scale=-1.0)
            nc.vector.tensor_scalar(out=vt[:], in0=vt[:], scalar1=0.0,
                                    scalar2=1.0, op0=OP.max, op1=OP.subtract)
            nc.vector.tensor_tensor(out=vt[:], in0=vt[:], in1=ex[:], op=OP.add)

            nc.sync.dma_start(outd[r0:r1, :], vt[:])


# ---------------------------------------------------------------------------
# Host-side driver
# ---------------------------------------------------------------------------

def _rotation(d=128):
    rng = np.random.default_rng(0)
    r, _ = np.linalg.qr(rng.standard_normal((d, d)).astype(np.float64))
    return r.astype(np.float32)


def prep_inputs(cfg: Cfg, features, neigh, emb, W, a):
    import ml_dtypes
    bf = ml_dtypes.bfloat16
    c = cfg
    D, K, E = c.d, c.k, c.e
    a = np.asarray(a, np.float32).reshape(-1)
    a_self, a_nb, a_edge = a[:D], a[D:2 * D], a[2 * D:]
    W = np.asarray(W, np.float32)
    R = _rotation(D)
    wext = np.concatenate(
        [W,
         ((1.0 - 2 * ALPHA) * (W @ a_self))[:, None],
         (2 * ALPHA * (W @ a_self))[:, None],
         W @ R,
         (W @ a_nb)[:, None]], axis=1)
    wext = np.ascontiguousarray(wext).astype(bf)
    rtb = np.ascontiguousarray(R.T.astype(bf))
    aer = np.ascontiguousarray(
        np.broadcast_to(np.tile(a_edge, K)[None, :], (128, K * E)).astype(bf))
    pidx, cidx = np.meshgrid(np.arange(128), np.arange(128), indexing="ij")
    msk_m = ((pidx // K) == (cidx % c.nsub)).astype(bf)

    neigh = np.asarray(neigh)
    padded = ((neigh // c.shard) * c.shard_pad + neigh % c.shard).astype(np.int64)

    features = np.ascontiguousarray(np.asarray(features, np.float32)).astype(bf)
    emb = np.asarray(emb, np.float32).reshape(c.n_total, K, E)

    def rowmap(cc_, r_):
        return cc_ * c.pairs + r_

    in_maps = []
    for ci in range(c.ncores):
        s0, s1 = ci * c.shard, (ci + 1) * c.shard
        pad = c.shard_pad - c.shard
        f = features[s0:s1]
        if pad:
            f = np.concatenate([f, np.zeros((pad, c.in_dim), bf)])
        em = emb[s0:s1]
        if pad:
            em = np.concatenate([em, np.zeros((pad, K, E), np.float32)])
        # packed emb layout: embP[ns*32+k, g*64+e] = emb[ns*32+g, k, e]
        em = em.reshape(c.tiles, 4, 32, K, E)          # [t, ns, g, k, e]
        em = em.transpose(0, 1, 3, 2, 4)               # [t, ns, k, g, e]
        em = np.ascontiguousarray(
            em.reshape(c.tiles * 128, K * E).astype(bf))

        nr = padded[s0:s1]
        if pad:
            nr = np.concatenate([nr, np.zeros((pad, K), np.int64)])
        nrt = nr.reshape(c.tiles, 4, K, K)          # [t, ns, g, k]
        st = nrt.transpose(0, 2, 1, 3)              # [t, g, ns, k]
        st = st.reshape(c.tiles, 128 * K)           # pos = g*128+32*ns+k
        pair_old = st // 2
        cc_ = pair_old // c.pairs
        r_ = pair_old % c.pairs
        pair = rowmap(cc_, r_).astype(np.int16)
        parity = (st & 1).astype(np.float32)
        pc = pair.reshape(c.tiles, c.chunks, CHUNK // 16, 16)
        wrapped = pc.transpose(0, 1, 3, 2)          # [t, chunk, 16, CHUNK//16]
        idx16 = np.ascontiguousarray(
            np.tile(wrapped, (1, 1, 8, 1))
            .transpose(0, 2, 1, 3)
            .reshape(c.tiles * 128, c.idx_cols))
        par_pk = parity.reshape(c.tiles, K, 128).transpose(0, 2, 1)
        parr = np.concatenate([par_pk, 1.0 - par_pk], axis=2)
        parr = np.ascontiguousarray(
            parr.reshape(c.tiles * 128, 2 * K).astype(bf))
        in_maps.append({
            "feat": np.ascontiguousarray(f),
            "wext": wext,
            "embd": em,
            "aer": aer,
            "msk": msk_m,
            "rtb": rtb,
            "idx": idx16,
            "parp": parr,
        })
    return in_maps


_CACHE = {}


def _get_compiled(key="full"):
    if key not in _CACHE:
        cfg = Cfg()
        _CACHE[key] = (cfg, build(cfg))
    return _CACHE[key]


def run(inputs, trace=False):
    cfg, nc = _get_compiled()
    in_maps = prep_inputs(cfg, inputs["features"], inputs["neigh"],
                          inputs["emb"], inputs["W"], inputs["a"])
    res = run_bass_kernel_spmd(nc, in_maps, list(range(cfg.ncores)),
                               trace=trace)
    outs = [res.results[ci]["outd"][:cfg.shard] for ci in range(cfg.ncores)]
    out = np.concatenate(outs, axis=0)
    return out, res.exec_time_ns


def kernel(**inputs):
    out, _ = run(inputs)
    return out


# revision 18
# speedup vs baseline: 1.0268x; 1.0268x over previous
"""EdgeAttentionAggregator Trainium2 kernel (8-core SPMD).

Reference computation (per node n, K=32 neighbors, D=128 out dim, E=64 edge):
    x = features @ W                                    [N, D]
    e[n,k]   = leakyrelu(x[n]@a_self + x[u]@a_nb + emb[n,k]@a_edge),  u=neigh[n,k]
    att      = softmax_k(e)
    h[n]     = sum_k att[n,k] * x[neigh[n,k]]
    h_e[n]   = sum_k att[n,k] * emb[n,k]
    out      = elu([x | h | h_e])                       [N, 2D+E]

Distribution: nodes sharded over 8 cores. Each core projects its shard,
a chunked AllGather replicates a PAIR-row table into every core's DRAM
(overlapping projection), and each core resolves neighbor reads with
dma_gather (mlp GPSIMD library) over 4 SWDGE queues.

The gather phase is descriptor-generation bound (one descriptor per edge,
Q7 core-pair per queue), so rows are packed to 512 bytes per pair:
  per node-half (256B): [hi: fp8e4m3(x@R) x128 | lo: fp8e4m3 residual
  dims 0:124 | s = x@a_nb as f32]
R is a host-chosen random rotation; quantization error of the 4 dims that
lack a residual is spread across all output dims, and the rotation is
undone for free by using R^T instead of the identity in the final PE
transpose of h. s rides exactly (f32) in the row.

Per-tile pipeline (packed edge layout: stream pos g*128 + 32*ns + k holds
edge (node 32*ns+g, k)): s_nb blended from the two parity s-slots, s_edge
on DVE in packed layout, one 32x32 block transpose to node-major for the
leakyrelu/softmax (ACT), block-diagonal parity-masked attention matrices,
h^T and h_e^T accumulated on the PE.

elu(v) = max(v,0) + exp(min(v,0)) - 1; lrelu(v) = 0.6v + 0.4|v| (slope 0.2).
"""

import numpy as np
from contextlib import ExitStack

import concourse.bass as bass
import concourse.tile as tile
from concourse import bacc, mybir
from concourse.tile import add_dep_helper
from concourse.bass_utils import run_bass_kernel_spmd
from concourse.masks import make_identity
from concourse import library_config

F32 = mybir.dt.float32
I16 = mybir.dt.int16
BF16 = mybir.dt.bfloat16
FP8 = mybir.dt.float8e4
AF = mybir.ActivationFunctionType
OP = mybir.AluOpType

ALPHA = 0.2   # leaky relu slope
CHUNK = 1024  # max dma_gather indices per call on this runtime
LO = 124      # residual-covered dims per node


class Cfg:
    def __init__(self, n_total=50000, k=32, in_dim=256, d=128, e=64, ncores=8):
        assert n_total % ncores == 0
        self.n_total = n_total
        self.k = k
        self.in_dim = in_dim
        self.d = d
        self.e = e
        self.ncores = ncores
        self.shard = n_total // ncores
        self.tiles = (self.shard + 127) // 128
        self.shard_pad = self.tiles * 128
        self.pairs = self.shard_pad // 2
        self.tbl_pairs = ncores * self.pairs
        assert self.tbl_pairs <= 32767
        self.row = 512            # fp8 units (bytes) per pair row
        self.half_row = 256
        self.sh_cols = 2 * d + 3  # f32: [x | ssl06 | ssl04 | x' | s_nb]
        self.out_cols = 2 * d + e
        self.nsub = 128 // k
        self.per_tile_idx = 128 * k
        self.chunks = self.per_tile_idx // CHUNK
        self.idx_cols = self.per_tile_idx // 16
        # AllGather chunk boundaries in pair rows (13/12/12/12 tiles)
        self.agb = [0, 832, 1600, 2368, self.pairs]
        assert all(b % 64 == 0 for b in self.agb)


def build(cfg: Cfg):
    c = cfg
    nc = bacc.Bacc("TRN2", target_bir_lowering=False, debug=False,
                   num_devices=c.ncores, num_swdge_queues=4)

    feat = nc.dram_tensor("feat", [c.shard_pad, c.in_dim], BF16,
                          kind="ExternalInput").ap()
    wext = nc.dram_tensor("wext", [c.in_dim, c.sh_cols], BF16,
                          kind="ExternalInput").ap()
    embd = nc.dram_tensor("embd", [c.shard_pad, c.k * c.e], BF16,
                          kind="ExternalInput").ap()
    aer = nc.dram_tensor("aer", [128, c.k * c.e], BF16,
                         kind="ExternalInput").ap()
    msk = nc.dram_tensor("msk", [128, 128], BF16, kind="ExternalInput").ap()
    rtb = nc.dram_tensor("rtb", [128, 128], BF16, kind="ExternalInput").ap()
    idx = nc.dram_tensor("idx", [c.tiles * 128, c.idx_cols], I16,
                         kind="ExternalInput").ap()
    parp = nc.dram_tensor("parp", [c.tiles * 128, 2 * c.k], BF16,
                          kind="ExternalInput").ap()
    outd = nc.dram_tensor("outd", [c.shard_pad, c.out_cols], F32,
                          kind="ExternalOutput").ap()
    shard_pair = nc.dram_tensor("shard_pair", [c.pairs, c.row // 2], BF16).ap()
    table = nc.dram_tensor("table", [c.tbl_pairs, c.row // 2], BF16).ap()

    with tile.TileContext(nc) as tc:
        _body(tc, c, feat, wext, embd, aer, msk, rtb, idx, parp, outd,
              shard_pair, table)

    nc.compile()
    return nc


def _body(tc, c: Cfg, feat, wext, embd, aer, msk, rtb, idx, parp, outd,
          shard_pair, table):
    nc = tc.nc
    D, K, E = c.d, c.k, c.e
    KE = K * E
    HR = c.half_row

    with ExitStack() as ctx:
        const = ctx.enter_context(tc.tile_pool(name="const", bufs=1))

        ident = const.tile([128, 128], F32, tag="ident")
        make_identity(nc, ident[:])
        identb = const.tile([128, 128], BF16, tag="identb")
        nc.vector.tensor_copy(identb[:], ident[:])
        rt_sb = const.tile([128, 128], BF16, tag="rt")
        nc.sync.dma_start(rt_sb[:], rtb[:, :])

        w_sb = []
        for ci in range(c.in_dim // 128):
            w = const.tile([128, c.sh_cols], BF16, tag=f"w{ci}")
            nc.sync.dma_start(w[:], wext[ci * 128:(ci + 1) * 128, :])
            w_sb.append(w)

        aer_sb = const.tile([128, KE], BF16, tag="aer")
        nc.sync.dma_start(aer_sb[:], aer[:, :])
        msk_sb = const.tile([128, 128], BF16, tag="msk")
        nc.sync.dma_start(msk_sb[:], msk[:, :])

        # resident per-tile f32 [x | ssl06 | ssl04] (130 cols per tile)
        xres = const.tile([128, c.tiles * 130], F32, tag="xres")

        n_sh = 3
        shtiles = [const.tile([128, HR], FP8, tag=f"sh{i}", name=f"sh{i}")
                   for i in range(n_sh)]

        lib = nc.gpsimd.load_library(library_config.mlp)

        # -------- Phase A: project own shard --------
        shard_writes = []
        with ExitStack() as actx:
            pa = actx.enter_context(tc.tile_pool(name="pa", bufs=3))
            psa = actx.enter_context(
                tc.tile_pool(name="psa", bufs=3, space="PSUM"))
            for t in range(c.tiles):
                ft = pa.tile([128, c.in_dim], BF16, tag="ft")
                nc.sync.dma_start(ft[:], feat[t * 128:(t + 1) * 128, :])
                ps_x = psa.tile([128, c.sh_cols], F32, tag="ps_x")
                nchunks = c.in_dim // 128
                for ci in range(nchunks):
                    ps_t = psa.tile([128, 128], BF16, tag="ps_t")
                    nc.tensor.transpose(ps_t[:], ft[:, ci * 128:(ci + 1) * 128],
                                        identb[:])
                    fT = pa.tile([128, 128], BF16, tag=f"fT{ci}")
                    if ci % 2 == 0:
                        nc.vector.tensor_copy(fT[:], ps_t[:])
                    else:
                        nc.scalar.copy(fT[:], ps_t[:])
                    nc.tensor.matmul(ps_x[:], lhsT=fT[:], rhs=w_sb[ci][:],
                                     start=(ci == 0), stop=(ci == nchunks - 1))
                nc.vector.tensor_copy(xres[:, t * 130:(t + 1) * 130],
                                      ps_x[:, 0:130])
                sh = shtiles[t % n_sh]
                # staging row per node: [hi fp8 x128 | lo fp8 x124 | s f32]
                nc.vector.tensor_copy(sh[:, 0:D], ps_x[:, 130:130 + D])
                nc.vector.tensor_tensor(out=sh[:, D:D + LO],
                                        in0=ps_x[:, 130:130 + LO],
                                        in1=sh[:, 0:LO], op=OP.subtract)
                shb = sh[:].bitcast(BF16)
                nc.vector.tensor_copy(shb[:, 126:127],
                                      ps_x[:, c.sh_cols - 1:c.sh_cols])
                nc.vector.tensor_tensor(out=shb[:, 127:128],
                                        in0=ps_x[:, c.sh_cols - 1:c.sh_cols],
                                        in1=shb[:, 126:127], op=OP.subtract)
                wr = nc.sync.dma_start(
                    shard_pair[t * 64:(t + 1) * 64, :]
                    .rearrange("r (p q) -> r p q", p=2),
                    sh[:].bitcast(BF16))
                shard_writes.append(wr)

        # -------- AllGather the pair-row table --------
        if c.ncores > 1:
            cc = nc.gpsimd.collective_compute(
                "AllGather", OP.bypass,
                replica_groups=[list(range(c.ncores))],
                ins=[shard_pair[:, :]],
                outs=[table[:, :]],
            )
        else:
            cc = nc.sync.dma_start(table[:, :], shard_pair[:, :])
        for wr in shard_writes:
            add_dep_helper(cc.ins, wr.ins, reason="table after shard write")
        ccs = [cc]

        # -------- Phase B: attention + aggregation --------
        pb = ctx.enter_context(tc.tile_pool(name="pb", bufs=2))
        psb = ctx.enter_context(tc.tile_pool(name="psb", bufs=2, space="PSUM"))

        for t in range(c.tiles):
            r0, r1 = t * 128, (t + 1) * 128
            idxt = pb.tile([128, c.idx_cols], I16, tag="idxt")
            nc.sync.dma_start(idxt[:], idx[r0:r1, :])
            part = pb.tile([128, 2 * K], BF16, tag="part")
            nc.sync.dma_start(part[:], parp[r0:r1, :])
            embt = pb.tile([128, KE], BF16, tag="embt")
            nc.sync.dma_start(embt[:], embd[r0:r1, :])

            gx = pb.tile([128, K * c.row // 2], BF16, tag="gx")
            nb_per = CHUNK // 128
            for ci in range(c.chunks):
                g1 = nc.gpsimd.dma_gather(
                    out_ap=gx[:, ci * nb_per * c.row // 2:
                              (ci + 1) * nb_per * c.row // 2]
                    .rearrange("p (b e) -> p b e", e=c.row // 2),
                    in_ap=table,
                    idxs_ap=idxt[:, ci * (CHUNK // 16):(ci + 1) * (CHUNK // 16)],
                    num_idxs=CHUNK,
                    num_idxs_reg=CHUNK,
                    elem_size=c.row // 2,
                    queue_num=(t * c.chunks + ci) % 4,
                )
                for cc in ccs:
                    add_dep_helper(g1.ins, cc.ins, reason="gather after table")
                add_dep_helper(g1.ins, lib.ins, reason="gather after lib")

            # s_nb: parity blend of the bf16 hi/lo s slots
            gxb = gx[:].rearrange("p (g w) -> p g w", w=HR)
            par_pk = part[:, 0:K]
            sev = pb.tile([128, K], F32, tag="sev")
            nc.vector.tensor_tensor(out=sev[:].unsqueeze(2),
                                    in0=gxb[:, :, 126:127],
                                    in1=gxb[:, :, 127:128], op=OP.add)
            sod = pb.tile([128, K], F32, tag="sod")
            nc.vector.tensor_tensor(out=sod[:].unsqueeze(2),
                                    in0=gxb[:, :, 254:255],
                                    in1=gxb[:, :, 255:256], op=OP.add)
            sdiff = pb.tile([128, K], F32, tag="sdiff")
            nc.vector.tensor_tensor(out=sdiff[:], in0=sod[:], in1=sev[:],
                                    op=OP.subtract)
            sdp = pb.tile([128, K], F32, tag="sdp")
            nc.vector.tensor_tensor(out=sdp[:], in0=sdiff[:], in1=par_pk,
                                    op=OP.mult)
            spk = pb.tile([128, K], F32, tag="spk")
            nc.vector.tensor_tensor(out=spk[:], in0=sev[:], in1=sdp[:],
                                    op=OP.add)

            # s_edge (packed layout): sum_e embP[p, g*64+e] * a_edge[e]
            prod = pb.tile([128, KE], BF16, tag="prod")
            nc.vector.tensor_tensor(out=prod[:], in0=embt[:], in1=aer_sb[:],
                                    op=OP.mult)
            sed = pb.tile([128, K], F32, tag="sed")
            nc.vector.tensor_reduce(
                out=sed[:], in_=prod[:].rearrange("p (k e) -> p k e", k=K),
                axis=mybir.AxisListType.X, op=OP.add)

            epk = pb.tile([128, K], F32, tag="epk")
            nc.vector.tensor_tensor(out=epk[:], in0=spk[:], in1=sed[:],
                                    op=OP.add)
            enm = pb.tile([128, K], F32, tag="enm")
            nc.vector.transpose(enm[:], epk[:])   # packed -> node-major

            # e = lrelu(v + s_self) = 0.6(v+s) + 0.4|v+s|
            ssl06 = xres[:, t * 130 + D: t * 130 + D + 1]
            ssl04 = xres[:, t * 130 + D + 1: t * 130 + D + 2]
            ab = pb.tile([128, K], F32, tag="ab")
            nc.scalar.activation(ab[:], enm[:], AF.Abs, bias=ssl04,
                                 scale=ALPHA * 2)
            e6 = pb.tile([128, K], F32, tag="e6")
            nc.vector.tensor_scalar(out=e6[:], in0=enm[:],
                                    scalar1=1.0 - ALPHA * 2, scalar2=ssl06,
                                    op0=OP.mult, op1=OP.add)
            elog = pb.tile([128, K], F32, tag="elog")
            nc.vector.tensor_tensor(out=elog[:], in0=e6[:], in1=ab[:],
                                    op=OP.add)

            p = pb.tile([128, K], BF16, tag="p")
            den = pb.tile([128, 1], F32, tag="den")
            nc.scalar.activation(p[:], elog[:], AF.Exp, accum_out=den[:])
            inv = pb.tile([128, 1], F32, tag="inv")
            nc.vector.reciprocal(inv[:], den[:])

            ppk = pb.tile([128, K], BF16, tag="ppk")
            nc.vector.transpose(ppk[:], p[:])     # node-major -> packed
            asb = pb.tile([128, 128], BF16, tag="asb")
            nc.vector.tensor_tensor(
                out=asb[:],
                in0=ppk[:].unsqueeze(2).to_broadcast([128, K, c.nsub]),
                in1=msk_sb[:], op=OP.mult)
            aev = pb.tile([128, 128], BF16, tag="aev")
            nc.vector.tensor_tensor(
                out=aev[:], in0=asb[:],
                in1=part[:, K:2 * K].unsqueeze(2).to_broadcast([128, K, c.nsub]),
                op=OP.mult)
            aod = pb.tile([128, 128], BF16, tag="aod")
            nc.vector.tensor_tensor(
                out=aod[:], in0=asb[:],
                in1=par_pk.unsqueeze(2).to_broadcast([128, K, c.nsub]),
                op=OP.mult)

            # h'^T on the PE: per block g accumulate hi/lo, even/odd
            # reconstruct x = hi + lo in bf16 once (DVE), then 2 matmuls/block
            gx8v = gx[:].bitcast(FP8).rearrange("p (b e) -> p b e", e=HR)
            xf = pb.tile([128, 2 * K * D], BF16, tag="xf")
            xfv = xf[:].rearrange("p (b e) -> p b e", e=D)
            nc.vector.tensor_tensor(out=xfv[:, :, 0:LO],
                                    in0=gx8v[:, :, 0:LO],
                                    in1=gx8v[:, :, D:D + LO], op=OP.add)
            nc.vector.tensor_copy(xfv[:, :, LO:D], gx8v[:, :, LO:D])
            htps = psb.tile([128, 128], F32, tag="htps")
            for g in range(K):
                o = g * c.nsub
                b0 = 2 * g * D
                nc.tensor.matmul(htps[:, o:o + c.nsub],
                                 lhsT=xf[:, b0:b0 + D],
                                 rhs=aev[:, o:o + c.nsub],
                                 start=True, stop=False)
                nc.tensor.matmul(htps[:, o:o + c.nsub],
                                 lhsT=xf[:, b0 + D:b0 + 2 * D],
                                 rhs=aod[:, o:o + c.nsub],
                                 start=False, stop=True)
            htsb = pb.tile([128, 128], BF16, tag="htsb")
            nc.scalar.copy(htsb[:].rearrange("p (m g) -> p m g", m=c.nsub),
                           htps[:].rearrange("p (g m) -> p m g", m=c.nsub))
            # un-rotate while transposing: h = (h'^T)^T @ R^T
            hps = psb.tile([128, D], F32, tag="hps")
            nc.tensor.matmul(hps[:], lhsT=htsb[:], rhs=rt_sb[:],
                             start=True, stop=True)

            # h_e^T on the PE: per block g, emb columns vs asb
            hetps = psb.tile([64, 128], F32, tag="hetps")
            for g in range(K):
                o = g * c.nsub
                nc.tensor.matmul(hetps[:, o:o + c.nsub],
                                 lhsT=embt[:, g * E:(g + 1) * E],
                                 rhs=asb[:, o:o + c.nsub],
                                 start=True, stop=True)
            hetsb = pb.tile([64, 128], BF16, tag="hetsb")
            nc.scalar.copy(hetsb[:].rearrange("p (m g) -> p m g", m=c.nsub),
                           hetps[:].rearrange("p (g m) -> p m g", m=c.nsub))
            heps = psb.tile([128, E], BF16, tag="heps")
            nc.tensor.transpose(heps[:], hetsb[:], identb[0:64, 0:64])

            vt = pb.tile([128, c.out_cols], F32, tag="vt")
            nc.scalar.copy(vt[:, 0:D], xres[:, t * 130:t * 130 + D])
            nc.scalar.activation(vt[:, D:2 * D], hps[:], AF.Copy, bias=0.0,
                                 scale=inv[:])
            nc.scalar.activation(vt[:, 2 * D:], heps[:], AF.Copy, bias=0.0,
                                 scale=inv[:])

            # elu(v) = max(v,0) + exp(-relu(-v)) - 1
            mn = pb.tile([128, c.out_cols], F32, tag="mn")
            nc.scalar.activation(mn[:], vt[:], AF.Relu, scale=-1.0)
            ex = pb.tile([128, c.out_cols], F32, tag="ex")
            nc.scalar.activation(ex[:], mn[:], AF.Exp, scale=-1.0)
            nc.vector.tensor_scalar(out=vt[:], in0=vt[:], scalar1=0.0,
                                    scalar2=1.0, op0=OP.max, op1=OP.subtract)
            nc.vector.tensor_tensor(out=vt[:], in0=vt[:], in1=ex[:], op=OP.add)

            nc.sync.dma_start(outd[r0:r1, :], vt[:])


# ---------------------------------------------------------------------------
# Host-side driver
# ---------------------------------------------------------------------------

def _rotation(d=128):
    rng = np.random.default_rng(0)
    r, _ = np.linalg.qr(rng.standard_normal((d, d)).astype(np.float64))
    return r.astype(np.float32)


def prep_inputs(cfg: Cfg, features, neigh, emb, W, a):
    import ml_dtypes
    bf = ml_dtypes.bfloat16
    c = cfg
    D, K, E = c.d, c.k, c.e
    a = np.asarray(a, np.float32).reshape(-1)
    a_self, a_nb, a_edge = a[:D], a[D:2 * D], a[2 * D:]
    W = np.asarray(W, np.float32)
    R = _rotation(D)
    wext = np.concatenate(
        [W,
         ((1.0 - 2 * ALPHA) * (W @ a_self))[:, None],
         (2 * ALPHA * (W @ a_self))[:, None],
         W @ R,
         (W @ a_nb)[:, None]], axis=1)
    wext = np.ascontiguousarray(wext).astype(bf)
    rtb = np.ascontiguousarray(R.T.astype(bf))
    aer = np.ascontiguousarray(
        np.broadcast_to(np.tile(a_edge, K)[None, :], (128, K * E)).astype(bf))
    pidx, cidx = np.meshgrid(np.arange(128), np.arange(128), indexing="ij")
    msk_m = ((pidx // K) == (cidx % c.nsub)).astype(bf)

    neigh = np.asarray(neigh)
    padded = ((neigh // c.shard) * c.shard_pad + neigh % c.shard).astype(np.int64)

    features = np.ascontiguousarray(np.asarray(features, np.float32)).astype(bf)
    emb = np.asarray(emb, np.float32).reshape(c.n_total, K, E)

    def rowmap(cc_, r_):
        return cc_ * c.pairs + r_

    in_maps = []
    for ci in range(c.ncores):
        s0, s1 = ci * c.shard, (ci + 1) * c.shard
        pad = c.shard_pad - c.shard
        f = features[s0:s1]
        if pad:
            f = np.concatenate([f, np.zeros((pad, c.in_dim), bf)])
        em = emb[s0:s1]
        if pad:
            em = np.concatenate([em, np.zeros((pad, K, E), np.float32)])
        # packed emb layout: embP[ns*32+k, g*64+e] = emb[ns*32+g, k, e]
        em = em.reshape(c.tiles, 4, 32, K, E)          # [t, ns, g, k, e]
        em = em.transpose(0, 1, 3, 2, 4)               # [t, ns, k, g, e]
        em = np.ascontiguousarray(
            em.reshape(c.tiles * 128, K * E).astype(bf))

        nr = padded[s0:s1]
        if pad:
            nr = np.concatenate([nr, np.zeros((pad, K), np.int64)])
        nrt = nr.reshape(c.tiles, 4, K, K)          # [t, ns, g, k]
        st = nrt.transpose(0, 2, 1, 3)              # [t, g, ns, k]
        st = st.reshape(c.tiles, 128 * K)           # pos = g*128+32*ns+k
        pair_old = st // 2
        cc_ = pair_old // c.pairs
        r_ = pair_old % c.pairs
        pair = rowmap(cc_, r_).astype(np.int16)
        parity = (st & 1).astype(np.float32)
        pc = pair.reshape(c.tiles, c.chunks, CHUNK // 16, 16)
        wrapped = pc.transpose(0, 1, 3, 2)          # [t, chunk, 16, CHUNK//16]
        idx16 = np.ascontiguousarray(
            np.tile(wrapped, (1, 1, 8, 1))
            .transpose(0, 2, 1, 3)
            .reshape(c.tiles * 128, c.idx_cols))
        par_pk = parity.reshape(c.tiles, K, 128).transpose(0, 2, 1)
        parr = np.concatenate([par_pk, 1.0 - par_pk], axis=2)
        parr = np.ascontiguousarray(
            parr.reshape(c.tiles * 128, 2 * K).astype(bf))
        in_maps.append({
            "feat": np.ascontiguousarray(f),
            "wext": wext,
            "embd": em,
            "aer": aer,
            "msk": msk_m,
            "rtb": rtb,
            "idx": idx16,
            "parp": parr,
        })
    return in_maps


_CACHE = {}


def _get_compiled(key="full"):
    if key not in _CACHE:
        cfg = Cfg()
        _CACHE[key] = (cfg, build(cfg))
    return _CACHE[key]


def run(inputs, trace=False):
    cfg, nc = _get_compiled()
    in_maps = prep_inputs(cfg, inputs["features"], inputs["neigh"],
                          inputs["emb"], inputs["W"], inputs["a"])
    res = run_bass_kernel_spmd(nc, in_maps, list(range(cfg.ncores)),
                               trace=trace)
    outs = [res.results[ci]["outd"][:cfg.shard] for ci in range(cfg.ncores)]
    out = np.concatenate(outs, axis=0)
    return out, res.exec_time_ns


def kernel(**inputs):
    out, _ = run(inputs)
    return out
